# revision 3
# baseline (speedup 1.0000x reference)
"""Llama decode block (single token) on 8 TRN2 NeuronCores, tensor-parallel.

v3: fp16 weights + fp8e4m3 residuals, column-form GEMV, host-contiguous
supertile layouts, pipelined MLP.

Precision (numpy-sim rel err ~1.1e-2 vs the 2e-2 gate; HW measured 8.6e-3):
  - w_q, w_o, w_ff1, w_ff2: fp16 + scaled-fp8 residual; activations split
    hi/lo fp16 (exact to ~2^-22) + fp8 copy for the residual pass.
  - w_k, w_v: fp16 only (error path is softmax-damped). K/V cache fp16.
  - K stored transposed per head so scores run on the PE with K stationary,
    landing token-major [128t, 4h] for the softmax.
  - RoPE as host-built 128x128 rotation matrices (PE matmul); 1/sqrt(d)
    folded into the q rotation.

Column-form GEMV: weight tile [128k, 128m] stationary (fp16 FWL), activation
pair [128, 2] (hi,lo) moving; kb-outer accumulation of all output blocks into
one PSUM bank; fp8 residual pass mirrors into a second bank; one DVE combine
per matrix. Weights are passed from the host already in supertile layout
[n_st, 128, *] so every DMA is a single contiguous segment per partition.

Stream order: wq -> kT/V (attention starts ~25us in) -> wk/wv -> wo -> AR1
-> wf1 (column-chunked: a-blocks complete per supertile and ff2 consumes them
kb-ordered, overlapping ff1/ff2) -> wf2. Dependency-gated DMAs ride the
scalar HWDGE queue so the sync queue streams weights without head-of-line
blocking. x/8 is folded into each AR so its output is the full residual.
"""

import math

import numpy as np
import ml_dtypes

import concourse.bass as bass
import concourse.mybir as mybir
import concourse.tile as tile
from concourse import bacc
from concourse import bass_utils

F32 = mybir.dt.float32
F16 = mybir.dt.float16
F8 = mybir.dt.float8e4
AF = mybir.ActivationFunctionType
ALU = mybir.AluOpType
AXL = mybir.AxisListType

HIDDEN = 4096
N_HEADS = 32
HEAD_DIM = 128
INTERM = 11008
KV_LEN = 4096
N_CORES = 8

HEADS_PC = N_HEADS // N_CORES          # 4
QKV_N = HEADS_PC * HEAD_DIM            # 512
FF_N = INTERM // N_CORES               # 1376
FF_NP = 1408                           # 11*128
KB = HIDDEN // 128                     # 32
T_TILES = KV_LEN // 128                # 32
SCALE = 1.0 / math.sqrt(HEAD_DIM)
F8NP = ml_dtypes.float8_e4m3

FF1_STS = [2, 2, 2, 2, 2, 1]           # ff1 col-chunk mb-blocks per supertile
WF2_STS = 8                            # wf2 column supertiles
WF2_CS = HIDDEN // WF2_STS             # 512 cols per st


def _emit(nc, tc):
    i = {}

    def din(name, shape, dt=F32):
        i[name] = nc.dram_tensor(name, list(shape), dt, kind="ExternalInput").ap()

    din("x", [HIDDEN])
    din("attn_norm", [HIDDEN])
    din("ffn_norm", [HIDDEN])
    din("ident32", [32, 32])
    din("ident128", [128, 128])
    din("rot_k", [128, 128])
    din("rot_q", [128, 128])
    din("consts", [1, 4])
    # pre-tiled weights: [n_st, 128, st_kb*csz] (contiguous per partition)
    din("wq16", [2, 128, 16 * QKV_N], F16)
    din("wq8", [2, 128, 16 * QKV_N], F8)
    din("wk16", [2, 128, 16 * QKV_N], F16)
    din("wv16", [2, 128, 16 * QKV_N], F16)
    din("kT16", [2, 128, HEADS_PC * 2048], F16)
    din("v16", [2, 128, 16 * QKV_N], F16)
    din("wo16", [2, 128, 2 * HIDDEN], F16)
    din("wo8", [2, 128, 2 * HIDDEN], F8)
    din("wf1_16", [128, KB * FF_NP], F16)    # col-chunked, concatenated sts
    din("wf1_8", [128, KB * FF_NP], F8)
    din("wf2_16", [WF2_STS, 128, 11 * WF2_CS], F16)
    din("wf2_8", [WF2_STS, 128, 11 * WF2_CS], F8)
    y = nc.dram_tensor("y", [HIDDEN], F32, kind="ExternalOutput").ap()

    with (
        tc.tile_pool(name="const", bufs=1) as cpool,
        tc.tile_pool(name="wA", bufs=6) as wA,
        tc.tile_pool(name="wB", bufs=3) as wB,
        tc.tile_pool(name="w8", bufs=8) as w8p,
        tc.tile_pool(name="sm", bufs=1) as sm,
        tc.tile_pool(name="scr", bufs=3) as scr,
        tc.tile_pool(name="psum", bufs=8, space="PSUM") as pp,
        tc.tile_pool(name="dram", bufs=1, space="DRAM") as dram,
    ):
        # ---- constants ----
        ones32 = cpool.tile([32, 1], F32)
        nc.vector.memset(ones32[:], 1.0)
        ones128 = cpool.tile([128, 1], F32)
        nc.vector.memset(ones128[:], 1.0)
        ones_r32 = cpool.tile([1, 32], F32)
        nc.vector.memset(ones_r32[:], 1.0)
        ones_r128 = cpool.tile([1, 128], F32)
        nc.vector.memset(ones_r128[:], 1.0)
        eps11 = cpool.tile([1, 1], F32)
        nc.vector.memset(eps11[:], 1e-6)
        ident32 = cpool.tile([32, 32], F32)
        nc.sync.dma_start(ident32[:], i["ident32"])
        ident128 = cpool.tile([128, 128], F32)
        nc.sync.dma_start(ident128[:], i["ident128"])
        rot_k = cpool.tile([128, 128], F32)
        nc.sync.dma_start(rot_k[:], i["rot_k"])
        rot_q = cpool.tile([128, 128], F32)
        nc.sync.dma_start(rot_q[:], i["rot_q"])
        consts = cpool.tile([1, 4], F32)
        nc.sync.dma_start(consts[:], i["consts"])

        inv_bc = []
        for j in range(4):
            ps = pp.tile([128, 1], F32, name=f"invb_{j}", tag="ps")
            nc.tensor.matmul(ps[:], ones_r128[:], consts[:, j:j + 1],
                             start=True, stop=True)
            t = cpool.tile([128, 1], F32, name=f"inv_{j}")
            nc.vector.tensor_copy(t[:], ps[:])
            inv_bc.append(t)
        inv_q, inv_o, inv_f1, inv_f2 = inv_bc

        dma_rr = [0]

        def wdma(dst, src_ap):
            eng = nc.sync if dma_rr[0] % 2 == 0 else nc.gpsimd
            dma_rr[0] += 1
            eng.dma_start(dst, src_ap)

        # ---- helpers ----
        def rmsnorm_cols(x_dram_ap, norm_sb, tag, eng=None):
            x_rows = sm.tile([32, 128], F32, name=f"x_rows_{tag}")
            (eng or nc.sync).dma_start(
                x_rows[:], x_dram_ap.rearrange("(a d) -> a d", a=32))
            sq = scr.tile([32, 128], F32, name=f"sq_{tag}", tag="sq")
            ssq = scr.tile([32, 1], F32, name=f"ssq_{tag}", tag="ssq")
            nc.scalar.activation(sq[:], x_rows[:], AF.Square, accum_out=ssq[:])
            ms_ps = pp.tile([1, 1], F32, name=f"ms_{tag}", tag="ps")
            nc.tensor.matmul(ms_ps[:], ones32[:], ssq[:], start=True, stop=True)
            rstd = scr.tile([1, 1], F32, name=f"rstd_{tag}", tag="rstd")
            nc.scalar.activation(rstd[:], ms_ps[:], AF.Sqrt,
                                 bias=eps11[:], scale=1.0 / HIDDEN)
            nc.vector.reciprocal(rstd[:], rstd[:])
            rstd_ps = pp.tile([32, 1], F32, name=f"rstdp_{tag}", tag="ps")
            nc.tensor.matmul(rstd_ps[:], ones_r32[:], rstd[:], start=True, stop=True)
            rstd32 = scr.tile([32, 1], F32, name=f"rstd32_{tag}", tag="rstd32")
            nc.vector.tensor_copy(rstd32[:], rstd_ps[:])
            h_rows = scr.tile([32, 128], F32, name=f"h_rows_{tag}", tag="hrows")
            nc.vector.tensor_tensor(h_rows[:], x_rows[:], norm_sb[:], ALU.mult)
            nc.vector.tensor_scalar_mul(h_rows[:], h_rows[:], rstd32[:])
            h_ps = pp.tile([128, 32], F32, name=f"hps_{tag}", tag="ps")
            nc.tensor.transpose(h_ps[:], h_rows[:], ident32[:])
            h_cols = sm.tile([128, 32], F32, name=f"h_cols_{tag}")
            nc.vector.tensor_copy(h_cols[:], h_ps[:])
            return x_rows, h_cols

        def split_hl(cols_ap, hl, h8, bs):
            """fp16 hi/lo + fp8 of cols_ap into block-slices of hl/h8."""
            nc.vector.tensor_copy(hl[:, bs, 0:1], cols_ap.unsqueeze(2))
            nc.vector.tensor_tensor(hl[:, bs, 1:2], cols_ap.unsqueeze(2),
                                    hl[:, bs, 0:1], ALU.subtract)
            nc.vector.tensor_copy(h8[:, bs], cols_ap)

        def gemv16(wname, n_mb, act_hl, lo, st_kb, n_kb, pool, tag):
            ncol = 2 if lo else 1
            ps = pp.tile([128, ncol * n_mb], F32, name=f"ps16_{tag}", tag="ps")
            n_st = (n_kb + st_kb - 1) // st_kb
            for st in range(n_st):
                kbs = list(range(st * st_kb, min((st + 1) * st_kb, n_kb)))
                wt = pool.tile([128, len(kbs), n_mb * 128], F16,
                               name=f"{wname}_t{st}", tag=pool.name)
                wdma(wt[:], i[wname][st].rearrange("p (b c) -> p b c", b=len(kbs)))
                for bi, kb in enumerate(kbs):
                    mv = act_hl[:, kb, :] if lo else act_hl[:, kb, 0:1]
                    for mb in range(n_mb):
                        nc.tensor.matmul(
                            ps[:, ncol * mb:ncol * mb + ncol],
                            wt[:, bi, mb * 128:(mb + 1) * 128],
                            mv,
                            start=(kb == 0 and mb == 0),
                            stop=(kb == n_kb - 1 and mb == n_mb - 1))
            return ps

        def gemv8(wname, n_mb, act8, st_kb, n_kb, tag):
            ps = pp.tile([128, n_mb], F32, name=f"ps8_{tag}", tag="ps")
            n_st = (n_kb + st_kb - 1) // st_kb
            for st in range(n_st):
                kbs = list(range(st * st_kb, min((st + 1) * st_kb, n_kb)))
                wt = w8p.tile([128, len(kbs), n_mb * 128], F8,
                              name=f"{wname}_t{st}", tag="w8")
                wdma(wt[:], i[wname][st].rearrange("p (b c) -> p b c", b=len(kbs)))
                for bi, kb in enumerate(kbs):
                    for mb in range(n_mb):
                        nc.tensor.matmul(
                            ps[:, mb:mb + 1],
                            wt[:, bi, mb * 128:(mb + 1) * 128],
                            act8[:, kb:kb + 1],
                            start=(kb == 0 and mb == 0),
                            stop=(kb == n_kb - 1 and mb == n_mb - 1))
            return ps

        def combine(ps16, ps8, inv_sc, n_mb, tag, lo=True):
            out = sm.tile([128, n_mb], F32, name=f"cmb_{tag}")
            if lo:
                p3 = ps16[:].rearrange("p (m two) -> p m two", two=2)
                nc.vector.tensor_reduce(out[:], p3, AXL.X, ALU.add)
            else:
                nc.vector.tensor_copy(out[:], ps16[:])
            if ps8 is not None:
                t = scr.tile([128, n_mb], F32, name=f"cmb8_{tag}", tag="cmb8")
                nc.vector.tensor_scalar_mul(t[:], ps8[:], inv_sc[:])
                nc.vector.tensor_add(out[:], out[:], t[:])
            return out

        def rope_cols(cols, rot, tag):
            ps = pp.tile([128, HEADS_PC], F32, name=f"rope_{tag}", tag="ps")
            nc.tensor.matmul(ps[:], rot[:], cols[:], start=True, stop=True)
            out = sm.tile([128, HEADS_PC], F32, name=f"rot_{tag}")
            nc.vector.tensor_copy(out[:], ps[:])
            return out

        # ---- rmsnorm #1, activation splits ----
        anorm = sm.tile([32, 128], F32, name="anorm")
        nc.sync.dma_start(anorm[:], i["attn_norm"].rearrange("(a d) -> a d", a=32))
        fnorm = sm.tile([32, 128], F32, name="fnorm")
        nc.sync.dma_start(fnorm[:], i["ffn_norm"].rearrange("(a d) -> a d", a=32))

        x_rows, h_cols = rmsnorm_cols(i["x"], anorm, "a")
        h_hl = sm.tile([128, KB, 2], F16, name="h_hl")
        h_8 = sm.tile([128, KB], F8, name="h_8")
        split_hl(h_cols[:], h_hl, h_8, slice(0, KB))
        x_ps = pp.tile([128, 32], F32, name="x_ps", tag="ps")
        nc.tensor.transpose(x_ps[:], x_rows[:], ident32[:])
        x_cols = sm.tile([128, 32], F32, name="x_cols")
        nc.vector.tensor_scalar_mul(x_cols[:], x_ps[:], 1.0 / N_CORES)

        # ---- q projection + rope ----
        ps_q = gemv16("wq16", HEADS_PC, h_hl, True, 16, KB, wA, "q")
        ps_q8 = gemv8("wq8", HEADS_PC, h_8, 16, KB, "q")
        q_cols = combine(ps_q, ps_q8, inv_q, HEADS_PC, "q")
        qr_cols = rope_cols(q_cols, rot_q, "q")
        q16 = sm.tile([128, HEADS_PC], F16, name="q16")
        nc.vector.tensor_copy(q16[:], qr_cols[:])

        # ---- attention over the KV cache ----
        o_psum = pp.tile([128, HEADS_PC], F32, name="o_psum", tag="ps")
        denom_acc = sm.tile([128, HEADS_PC], F32, name="denom_acc")
        nc.vector.memset(denom_acc[:], 0.0)

        kT_t = []
        for st in range(2):
            kt = wA.tile([128, HEADS_PC, 2048], F16, name=f"kT_{st}", tag="wA")
            wdma(kt[:], i["kT16"][st].rearrange("p (h t) -> p h t", h=HEADS_PC))
            kT_t.append(kt)
        v_t = []
        for st in range(2):
            vt = wA.tile([128, 16, QKV_N], F16, name=f"v_{st}", tag="wA")
            wdma(vt[:], i["v16"][st].rearrange("p (b c) -> p b c", b=16))
            v_t.append(vt)

        def emit_scores(tt):
            st, loc = tt // 16, tt % 16
            s_ps = pp.tile([128, HEADS_PC], F32, name="s_ps", tag="ps")
            for h in range(HEADS_PC):
                nc.tensor.matmul(
                    s_ps[:, h:h + 1],
                    kT_t[st][:, h, loc * 128:(loc + 1) * 128],
                    q16[:, h:h + 1],
                    start=(h == 0), stop=(h == HEADS_PC - 1))
            expt = scr.tile([128, HEADS_PC], F32, name="expt", tag="expt")
            nc.scalar.activation(expt[:], s_ps[:], AF.Exp)
            nc.vector.tensor_add(denom_acc[:], denom_acc[:], expt[:])
            expt16 = scr.tile([128, HEADS_PC], F16, name="expt16", tag="expt16")
            nc.vector.tensor_copy(expt16[:], expt[:])
            return expt16

        pending = emit_scores(0)
        for tt in range(T_TILES):
            st, loc = tt // 16, tt % 16
            expt16 = pending
            if tt + 1 < T_TILES:
                pending = emit_scores(tt + 1)
            for h in range(HEADS_PC):
                nc.tensor.matmul(
                    o_psum[:, h:h + 1],
                    v_t[st][:, loc, h * 128:(h + 1) * 128],
                    expt16[:, h:h + 1],
                    start=(tt == 0 and h == 0),
                    stop=(tt == T_TILES - 1 and h == HEADS_PC - 1))

        # ---- k/v of the current token ----
        ps_k = gemv16("wk16", HEADS_PC, h_hl, False, 16, KB, wA, "k")
        k_cols = combine(ps_k, None, None, HEADS_PC, "k", lo=False)
        kr_cols = rope_cols(k_cols, rot_k, "k")
        ps_v = gemv16("wv16", HEADS_PC, h_hl, False, 16, KB, wA, "v")
        v_cols = combine(ps_v, None, None, HEADS_PC, "v", lo=False)

        qk = scr.tile([128, HEADS_PC], F32, name="qk", tag="qk")
        nc.vector.tensor_tensor(qk[:], qr_cols[:], kr_cols[:], ALU.mult)
        snew_ps = pp.tile([1, HEADS_PC], F32, name="snew", tag="ps")
        nc.tensor.matmul(snew_ps[:], ones128[:], qk[:], start=True, stop=True)
        e_new = sm.tile([1, HEADS_PC], F32, name="e_new")
        nc.scalar.activation(e_new[:], snew_ps[:], AF.Exp)

        denom_ps = pp.tile([1, HEADS_PC], F32, name="denom_ps", tag="ps")
        nc.tensor.matmul(denom_ps[:], ones128[:], denom_acc[:],
                         start=True, stop=True)
        denom = sm.tile([1, HEADS_PC], F32, name="denom")
        nc.vector.tensor_copy(denom[:], denom_ps[:])
        nc.vector.tensor_add(denom[:], denom[:], e_new[:])
        nc.vector.reciprocal(denom[:], denom[:])
        rec_ps = pp.tile([128, HEADS_PC], F32, name="rec_ps", tag="ps")
        nc.tensor.matmul(rec_ps[:], ones_r128[:], denom[:], start=True, stop=True)
        enew_ps = pp.tile([128, HEADS_PC], F32, name="enew_ps", tag="ps")
        nc.tensor.matmul(enew_ps[:], ones_r128[:], e_new[:], start=True, stop=True)

        o_sb = sm.tile([128, HEADS_PC], F32, name="o_sb")
        nc.vector.tensor_tensor(o_sb[:], enew_ps[:], v_cols[:], ALU.mult)
        nc.vector.tensor_add(o_sb[:], o_sb[:], o_psum[:])
        nc.vector.tensor_tensor(o_sb[:], o_sb[:], rec_ps[:], ALU.mult)
        o_hl = sm.tile([128, HEADS_PC, 2], F16, name="o_hl")
        o_8 = sm.tile([128, HEADS_PC], F8, name="o_8")
        split_hl(o_sb[:], o_hl, o_8, slice(0, HEADS_PC))

        # ---- o @ w_o + x/8 -> AR1 ----
        ps_wo = gemv16("wo16", KB, o_hl, True, 2, HEADS_PC, wA, "wo")
        ps_wo8 = gemv8("wo8", KB, o_8, 2, HEADS_PC, "wo")
        y1_cols = combine(ps_wo, ps_wo8, inv_o, KB, "wo")
        nc.vector.tensor_add(y1_cols[:], y1_cols[:], x_cols[:])
        y1r_ps = pp.tile([32, 128], F32, name="y1r", tag="ps")
        nc.tensor.transpose(y1r_ps[:], y1_cols[:], ident128[:])
        y1_rows = sm.tile([32, 128], F32, name="y1_rows")
        nc.vector.tensor_copy(y1_rows[:], y1r_ps[:])

        ar1_in = dram.tile([HIDDEN], F32, name="ar1_in")
        ar1_out = nc.dram_tensor("ar1_out", [HIDDEN], F32, kind="Internal",
                                 addr_space="Shared").ap()
        nc.scalar.dma_start(ar1_in[:].rearrange("(a d) -> a d", a=32), y1_rows[:])
        nc.gpsimd.collective_compute(
            "AllReduce", ALU.add,
            replica_groups=[list(range(N_CORES))],
            ins=[ar1_in[:].opt()], outs=[ar1_out[:].opt()],
        )

        # ---- MLP ----
        x2_rows, h2_cols = rmsnorm_cols(ar1_out[:], fnorm, "b", eng=nc.scalar)
        h2_hl = sm.tile([128, KB, 2], F16, name="h2_hl")
        h2_8 = sm.tile([128, KB], F8, name="h2_8")
        split_hl(h2_cols[:], h2_hl, h2_8, slice(0, KB))
        x2_ps = pp.tile([128, 32], F32, name="x2_ps", tag="ps")
        nc.tensor.transpose(x2_ps[:], x2_rows[:], ident32[:])
        x2_cols = sm.tile([128, 32], F32, name="x2_cols")
        nc.vector.tensor_scalar_mul(x2_cols[:], x2_ps[:], 1.0 / N_CORES)

        # ff1: column-chunked supertiles; a-blocks complete per chunk
        a_hl = sm.tile([128, 11, 2], F16, name="a_hl")
        a_8 = sm.tile([128, 11], F8, name="a_8")
        mb0 = 0
        off = 0
        for sti, nb in enumerate(FF1_STS):
            csz = nb * 128
            wt = wA.tile([128, KB, csz], F16, name=f"wf1_t{sti}", tag="wA")
            wdma(wt[:], i["wf1_16"][:, off:off + KB * csz].rearrange(
                "p (b c) -> p b c", b=KB))
            wt8 = w8p.tile([128, KB, csz], F8, name=f"wf1_8t{sti}", tag="w8")
            wdma(wt8[:], i["wf1_8"][:, off:off + KB * csz].rearrange(
                "p (b c) -> p b c", b=KB))
            off += KB * csz
            ps = pp.tile([128, 2 * nb], F32, name=f"ps16_f1_{sti}", tag="ps")
            ps8 = pp.tile([128, nb], F32, name=f"ps8_f1_{sti}", tag="ps")
            for kb in range(KB):
                for mbl in range(nb):
                    nc.tensor.matmul(
                        ps[:, 2 * mbl:2 * mbl + 2],
                        wt[:, kb, mbl * 128:(mbl + 1) * 128],
                        h2_hl[:, kb, :],
                        start=(kb == 0 and mbl == 0),
                        stop=(kb == KB - 1 and mbl == nb - 1))
            for kb in range(KB):
                for mbl in range(nb):
                    nc.tensor.matmul(
                        ps8[:, mbl:mbl + 1],
                        wt8[:, kb, mbl * 128:(mbl + 1) * 128],
                        h2_8[:, kb:kb + 1],
                        start=(kb == 0 and mbl == 0),
                        stop=(kb == KB - 1 and mbl == nb - 1))
            a_blk = combine(ps, ps8, inv_f1, nb, f"f1_{sti}")
            sig = scr.tile([128, nb], F32, name=f"sig_{sti}", tag="sig")
            nc.scalar.activation(sig[:], a_blk[:], AF.Sigmoid)
            nc.vector.tensor_tensor(a_blk[:], a_blk[:], sig[:], ALU.mult)
            if sti == len(FF1_STS) - 1:
                nc.vector.memset(a_blk[96:128, nb - 1:nb], 0.0)
            split_hl(a_blk[:], a_hl, a_8, slice(mb0, mb0 + nb))
            mb0 += nb

        # ff2: column-chunked sts, kb-outer inside (consumes a-blocks in order)
        ps_f2 = pp.tile([128, 2 * KB], F32, name="ps16_f2", tag="ps")
        ps_f28 = pp.tile([128, KB], F32, name="ps8_f2", tag="ps")
        nmb = WF2_CS // 128                          # 4
        for st in range(WF2_STS):
            wt = wB.tile([128, 11, WF2_CS], F16, name=f"wf2_t{st}", tag="wB")
            wdma(wt[:], i["wf2_16"][st].rearrange("p (b c) -> p b c", b=11))
            wt8 = w8p.tile([128, 11, WF2_CS], F8, name=f"wf2_8t{st}", tag="w8")
            wdma(wt8[:], i["wf2_8"][st].rearrange("p (b c) -> p b c", b=11))
            for kb in range(11):
                for mbl in range(nmb):
                    mb = st * nmb + mbl
                    nc.tensor.matmul(
                        ps_f2[:, 2 * mb:2 * mb + 2],
                        wt[:, kb, mbl * 128:(mbl + 1) * 128],
                        a_hl[:, kb, :],
                        start=(st == 0 and kb == 0 and mbl == 0),
                        stop=(st == WF2_STS - 1 and kb == 10 and mbl == nmb - 1))
            for kb in range(11):
                for mbl in range(nmb):
                    mb = st * nmb + mbl
                    nc.tensor.matmul(
                        ps_f28[:, mb:mb + 1],
                        wt8[:, kb, mbl * 128:(mbl + 1) * 128],
                        a_8[:, kb:kb + 1],
                        start=(st == 0 and kb == 0 and mbl == 0),
                        stop=(st == WF2_STS - 1 and kb == 10 and mbl == nmb - 1))
        y2_cols = combine(ps_f2, ps_f28, inv_f2, KB, "f2")
        nc.vector.tensor_add(y2_cols[:], y2_cols[:], x2_cols[:])
        y2r_ps = pp.tile([32, 128], F32, name="y2r", tag="ps")
        nc.tensor.transpose(y2r_ps[:], y2_cols[:], ident128[:])
        y2_rows = sm.tile([32, 128], F32, name="y2_rows")
        nc.vector.tensor_copy(y2_rows[:], y2r_ps[:])

        ar2_in = dram.tile([HIDDEN], F32, name="ar2_in")
        ar2_out = nc.dram_tensor("ar2_out", [HIDDEN], F32, kind="Internal",
                                 addr_space="Shared").ap()
        nc.scalar.dma_start(ar2_in[:].rearrange("(a d) -> a d", a=32), y2_rows[:])
        nc.gpsimd.collective_compute(
            "AllReduce", ALU.add,
            replica_groups=[list(range(N_CORES))],
            ins=[ar2_in[:].opt()], outs=[ar2_out[:].opt()],
        )
        nc.scalar.dma_start(y[:], ar2_out[:])


_BUILT = None


def _build():
    global _BUILT
    if _BUILT is None:
        nc = bacc.Bacc("TRN2", target_bir_lowering=False, debug=False,
                       num_devices=N_CORES)
        with tile.TileContext(nc) as tc:
            _emit(nc, tc)
        nc.compile()
        _BUILT = nc
    return _BUILT


def _rot_mats(sin, cos):
    R = np.zeros((128, 128), np.float32)
    for d in range(64):
        R[d, d] = cos[d]
        R[d + 64, d] = -sin[d]
        R[d + 64, d + 64] = cos[d]
        R[d, d + 64] = sin[d]
    return R, (R * SCALE).astype(np.float32)


def _split16_8(a):
    a = np.ascontiguousarray(a, dtype=np.float32)
    w16 = a.astype(np.float16)
    r = a - w16.astype(np.float32)
    m = float(np.abs(r).max())
    c = 192.0 / m if m > 0 else 1.0
    w8 = (r * c).astype(F8NP)
    return w16, w8, np.float32(1.0 / c)


def _row_tiles(a, st_kb):
    """[n_kb*128, C] -> [n_st, 128, st_kb*C] (per-partition contiguous)."""
    rows, C = a.shape
    n_kb = rows // 128
    n_st = n_kb // st_kb
    return np.ascontiguousarray(
        a.reshape(n_st, st_kb, 128, C).transpose(0, 2, 1, 3).reshape(
            n_st, 128, st_kb * C))


def _col_tiles_flat(a, sts):
    """[n_kb*128, C] col-chunked by sts mb-blocks -> [128, n_kb*C] concat."""
    n_kb = a.shape[0] // 128
    out = []
    mb0 = 0
    for nb in sts:
        csz = nb * 128
        blk = a[:, mb0 * 128:mb0 * 128 + csz]
        out.append(blk.reshape(n_kb, 128, csz).transpose(1, 0, 2).reshape(
            128, n_kb * csz))
        mb0 += nb
    return np.ascontiguousarray(np.concatenate(out, axis=1))


def _shard(inputs):
    f = lambda a: np.ascontiguousarray(np.asarray(a, dtype=np.float32))
    x = f(inputs["x"])
    attn_norm = f(inputs["attn_norm"])
    ffn_norm = f(inputs["ffn_norm"])
    pos = int(np.asarray(inputs["pos"]))
    sin = f(inputs["sin_cache"][pos])
    cos = f(inputs["cos_cache"][pos])
    wq, wk, wv = f(inputs["w_q"]), f(inputs["w_k"]), f(inputs["w_v"])
    wo, wf1, wf2 = f(inputs["w_o"]), f(inputs["w_ff1"]), f(inputs["w_ff2"])
    kc = f(inputs["k_cache"])
    vc = f(inputs["v_cache"])
    rot_k, rot_q = _rot_mats(sin, cos)

    in_maps = []
    for c in range(N_CORES):
        qs = slice(c * QKV_N, (c + 1) * QKV_N)
        fs = slice(c * FF_N, (c + 1) * FF_N)
        hs = slice(c * HEADS_PC, (c + 1) * HEADS_PC)
        wq16, wq8, iq = _split16_8(wq[:, qs])
        wo16, wo8, io = _split16_8(wo[qs, :])
        wf1p = np.zeros((HIDDEN, FF_NP), np.float32)
        wf1p[:, :FF_N] = wf1[:, fs]
        wf116, wf18, if1 = _split16_8(wf1p)
        wf2p = np.zeros((FF_NP, HIDDEN), np.float32)
        wf2p[:FF_N] = wf2[fs, :]
        wf216, wf28, if2 = _split16_8(wf2p)
        # kT: [h*128+d, t]; supertiles over t (2 halves), contiguous layout:
        # [st, 128(d), 4(h)*2048(t)]
        kT = kc[:, hs, :].transpose(1, 2, 0).astype(np.float16)  # [4,128,4096]
        kT_tiled = np.ascontiguousarray(
            kT.reshape(HEADS_PC, 128, 2, 2048).transpose(2, 1, 0, 3).reshape(
                2, 128, HEADS_PC * 2048))
        v2d = vc[:, hs, :].reshape(KV_LEN, QKV_N).astype(np.float16)
        # wf2: column supertiles of WF2_CS cols, 11 kb rows each
        def wf2_tiles(a):
            # [FF_NP, HIDDEN] -> [WF2_STS, 128, 11*WF2_CS]
            t = a.reshape(11, 128, WF2_STS, WF2_CS).transpose(2, 1, 0, 3)
            return np.ascontiguousarray(
                t.reshape(WF2_STS, 128, 11 * WF2_CS))
        in_maps.append({
            "x": x,
            "attn_norm": attn_norm,
            "ffn_norm": ffn_norm,
            "ident32": np.eye(32, dtype=np.float32),
            "ident128": np.eye(128, dtype=np.float32),
            "rot_k": rot_k,
            "rot_q": rot_q,
            "consts": np.array([[iq, io, if1, if2]], np.float32),
            "wq16": _row_tiles(wq16, 16),
            "wq8": _row_tiles(wq8, 16),
            "wk16": _row_tiles(wk[:, qs].astype(np.float16), 16),
            "wv16": _row_tiles(wv[:, qs].astype(np.float16), 16),
            "kT16": kT_tiled,
            "v16": _row_tiles(v2d, 16),
            "wo16": _row_tiles(wo16, 2),
            "wo8": _row_tiles(wo8, 2),
            "wf1_16": _col_tiles_flat(wf116, FF1_STS),
            "wf1_8": _col_tiles_flat(wf18, FF1_STS),
            "wf2_16": wf2_tiles(wf216),
            "wf2_8": wf2_tiles(wf28),
        })
    return in_maps


def kernel(**inputs):
    nc = _build()
    in_maps = _shard(inputs)
    res = bass_utils.run_bass_kernel_spmd(
        nc, in_maps, core_ids=list(range(N_CORES)))
    return res.results[0]["y"]


# revision 4
# speedup vs baseline: 1.0417x; 1.0417x over previous
"""Llama decode block (single token) on 8 TRN2 NeuronCores, tensor-parallel.

v3: fp16 weights + fp8e4m3 residuals, column-form GEMV, host-contiguous
supertile layouts, pipelined MLP.

Precision (numpy-sim rel err ~1.1e-2 vs the 2e-2 gate; HW measured 8.6e-3):
  - w_q, w_o, w_ff1, w_ff2: fp16 + scaled-fp8 residual; activations split
    hi/lo fp16 (exact to ~2^-22) + fp8 copy for the residual pass.
  - w_k, w_v: fp16 only (error path is softmax-damped). K/V cache fp16.
  - K stored transposed per head so scores run on the PE with K stationary,
    landing token-major [128t, 4h] for the softmax.
  - RoPE as host-built 128x128 rotation matrices (PE matmul); 1/sqrt(d)
    folded into the q rotation.

Column-form GEMV: weight tile [128k, 128m] stationary (fp16 FWL), activation
pair [128, 2] (hi,lo) moving; kb-outer accumulation of all output blocks into
one PSUM bank; fp8 residual pass mirrors into a second bank; one DVE combine
per matrix. Weights are passed from the host already in supertile layout
[n_st, 128, *] so every DMA is a single contiguous segment per partition.

Stream order: wq -> kT/V (attention starts ~25us in) -> wk/wv -> wo -> AR1
-> wf1 (column-chunked: a-blocks complete per supertile and ff2 consumes them
kb-ordered, overlapping ff1/ff2) -> wf2. Dependency-gated DMAs ride the
scalar HWDGE queue so the sync queue streams weights without head-of-line
blocking. x/8 is folded into each AR so its output is the full residual.
"""

import math

import numpy as np
import ml_dtypes

import concourse.bass as bass
import concourse.mybir as mybir
import concourse.tile as tile
from concourse import bacc
from concourse import bass_utils

F32 = mybir.dt.float32
F16 = mybir.dt.float16
F8 = mybir.dt.float8e4
AF = mybir.ActivationFunctionType
ALU = mybir.AluOpType
AXL = mybir.AxisListType

HIDDEN = 4096
N_HEADS = 32
HEAD_DIM = 128
INTERM = 11008
KV_LEN = 4096
N_CORES = 8

HEADS_PC = N_HEADS // N_CORES          # 4
QKV_N = HEADS_PC * HEAD_DIM            # 512
FF_N = INTERM // N_CORES               # 1376
FF_NP = 1408                           # 11*128
KB = HIDDEN // 128                     # 32
T_TILES = KV_LEN // 128                # 32
SCALE = 1.0 / math.sqrt(HEAD_DIM)
F8NP = ml_dtypes.float8_e4m3

FF1_STS = [2, 2, 2, 2, 2, 1]           # ff1 col-chunk mb-blocks per supertile
WF2_STS = 8                            # wf2 column supertiles
WF2_CS = HIDDEN // WF2_STS             # 512 cols per st


def _emit(nc, tc):
    i = {}

    def din(name, shape, dt=F32):
        i[name] = nc.dram_tensor(name, list(shape), dt, kind="ExternalInput").ap()

    din("x", [HIDDEN])
    din("attn_norm", [HIDDEN])
    din("ffn_norm", [HIDDEN])
    din("ident32", [32, 32])
    din("ident128", [128, 128])
    din("rot_k", [128, 128])
    din("rot_q", [128, 128])
    din("consts", [1, 4])
    # pre-tiled weights: [n_st, 128, st_kb*csz] (contiguous per partition)
    din("wq16", [2, 128, 16 * QKV_N], F16)
    din("wq8", [2, 128, 16 * QKV_N], F8)
    din("wk16", [2, 128, 16 * QKV_N], F16)
    din("wv16", [2, 128, 16 * QKV_N], F16)
    din("kT16", [2, 128, HEADS_PC * 2048], F16)
    din("v16", [2, 128, 16 * QKV_N], F16)
    din("wo16", [2, 128, 2 * HIDDEN], F16)
    din("wo8", [2, 128, 2 * HIDDEN], F8)
    din("wf1_16", [128, KB * FF_NP], F16)    # col-chunked, concatenated sts
    din("wf1_8", [128, KB * FF_NP], F8)
    din("wf2_16", [WF2_STS, 128, 11 * WF2_CS], F16)
    din("wf2_8", [WF2_STS, 128, 11 * WF2_CS], F8)
    y = nc.dram_tensor("y", [HIDDEN], F32, kind="ExternalOutput").ap()

    with (
        tc.tile_pool(name="const", bufs=1) as cpool,
        tc.tile_pool(name="wA", bufs=6) as wA,
        tc.tile_pool(name="wB", bufs=3) as wB,
        tc.tile_pool(name="w8", bufs=8) as w8p,
        tc.tile_pool(name="sm", bufs=1) as sm,
        tc.tile_pool(name="scr", bufs=3) as scr,
        tc.tile_pool(name="psum", bufs=8, space="PSUM") as pp,
        tc.tile_pool(name="dram", bufs=1, space="DRAM") as dram,
    ):
        # ---- constants ----
        ones32 = cpool.tile([32, 1], F32)
        nc.vector.memset(ones32[:], 1.0)
        ones128 = cpool.tile([128, 1], F32)
        nc.vector.memset(ones128[:], 1.0)
        ones_r32 = cpool.tile([1, 32], F32)
        nc.vector.memset(ones_r32[:], 1.0)
        ones_r128 = cpool.tile([1, 128], F32)
        nc.vector.memset(ones_r128[:], 1.0)
        eps11 = cpool.tile([1, 1], F32)
        nc.vector.memset(eps11[:], 1e-6)
        ident32 = cpool.tile([32, 32], F32)
        nc.sync.dma_start(ident32[:], i["ident32"])
        ident128 = cpool.tile([128, 128], F32)
        nc.sync.dma_start(ident128[:], i["ident128"])
        rot_k = cpool.tile([128, 128], F32)
        nc.sync.dma_start(rot_k[:], i["rot_k"])
        rot_q = cpool.tile([128, 128], F32)
        nc.sync.dma_start(rot_q[:], i["rot_q"])
        consts = cpool.tile([1, 4], F32)
        nc.sync.dma_start(consts[:], i["consts"])

        inv_bc = []
        for j in range(4):
            ps = pp.tile([128, 1], F32, name=f"invb_{j}", tag="ps")
            nc.tensor.matmul(ps[:], ones_r128[:], consts[:, j:j + 1],
                             start=True, stop=True)
            t = cpool.tile([128, 1], F32, name=f"inv_{j}")
            nc.vector.tensor_copy(t[:], ps[:])
            inv_bc.append(t)
        inv_q, inv_o, inv_f1, inv_f2 = inv_bc

        dma_rr = [0]

        def wdma(dst, src_ap):
            eng = nc.sync if dma_rr[0] % 2 == 0 else nc.gpsimd
            dma_rr[0] += 1
            eng.dma_start(dst, src_ap)

        # ---- helpers ----
        def rmsnorm_cols(x_dram_ap, norm_sb, tag, eng=None):
            x_rows = sm.tile([32, 128], F32, name=f"x_rows_{tag}")
            (eng or nc.sync).dma_start(
                x_rows[:], x_dram_ap.rearrange("(a d) -> a d", a=32))
            sq = scr.tile([32, 128], F32, name=f"sq_{tag}", tag="sq")
            ssq = scr.tile([32, 1], F32, name=f"ssq_{tag}", tag="ssq")
            nc.scalar.activation(sq[:], x_rows[:], AF.Square, accum_out=ssq[:])
            ms_ps = pp.tile([1, 1], F32, name=f"ms_{tag}", tag="ps")
            nc.tensor.matmul(ms_ps[:], ones32[:], ssq[:], start=True, stop=True)
            rstd = scr.tile([1, 1], F32, name=f"rstd_{tag}", tag="rstd")
            nc.scalar.activation(rstd[:], ms_ps[:], AF.Sqrt,
                                 bias=eps11[:], scale=1.0 / HIDDEN)
            nc.vector.reciprocal(rstd[:], rstd[:])
            rstd_ps = pp.tile([32, 1], F32, name=f"rstdp_{tag}", tag="ps")
            nc.tensor.matmul(rstd_ps[:], ones_r32[:], rstd[:], start=True, stop=True)
            rstd32 = scr.tile([32, 1], F32, name=f"rstd32_{tag}", tag="rstd32")
            nc.vector.tensor_copy(rstd32[:], rstd_ps[:])
            h_rows = scr.tile([32, 128], F32, name=f"h_rows_{tag}", tag="hrows")
            nc.vector.tensor_tensor(h_rows[:], x_rows[:], norm_sb[:], ALU.mult)
            nc.vector.tensor_scalar_mul(h_rows[:], h_rows[:], rstd32[:])
            h_ps = pp.tile([128, 32], F32, name=f"hps_{tag}", tag="ps")
            nc.tensor.transpose(h_ps[:], h_rows[:], ident32[:])
            h_cols = sm.tile([128, 32], F32, name=f"h_cols_{tag}")
            nc.vector.tensor_copy(h_cols[:], h_ps[:])
            return x_rows, h_cols

        def split_hl(cols_ap, hl, h8, bs):
            """fp16 hi/lo + fp8 of cols_ap into block-slices of hl/h8."""
            nc.vector.tensor_copy(hl[:, bs, 0:1], cols_ap.unsqueeze(2))
            nc.vector.tensor_tensor(hl[:, bs, 1:2], cols_ap.unsqueeze(2),
                                    hl[:, bs, 0:1], ALU.subtract)
            nc.vector.tensor_copy(h8[:, bs], cols_ap)

        def gemv16(wname, n_mb, act_hl, lo, st_kb, n_kb, pool, tag):
            ncol = 2 if lo else 1
            ps = pp.tile([128, ncol * n_mb], F32, name=f"ps16_{tag}", tag="ps")
            n_st = (n_kb + st_kb - 1) // st_kb
            for st in range(n_st):
                kbs = list(range(st * st_kb, min((st + 1) * st_kb, n_kb)))
                wt = pool.tile([128, len(kbs), n_mb * 128], F16,
                               name=f"{wname}_t{st}", tag=pool.name)
                wdma(wt[:], i[wname][st].rearrange("p (b c) -> p b c", b=len(kbs)))
                for bi, kb in enumerate(kbs):
                    mv = act_hl[:, kb, :] if lo else act_hl[:, kb, 0:1]
                    for mb in range(n_mb):
                        nc.tensor.matmul(
                            ps[:, ncol * mb:ncol * mb + ncol],
                            wt[:, bi, mb * 128:(mb + 1) * 128],
                            mv,
                            start=(kb == 0 and mb == 0),
                            stop=(kb == n_kb - 1 and mb == n_mb - 1))
            return ps

        def gemv8(wname, n_mb, act8, st_kb, n_kb, tag):
            ps = pp.tile([128, n_mb], F32, name=f"ps8_{tag}", tag="ps")
            n_st = (n_kb + st_kb - 1) // st_kb
            for st in range(n_st):
                kbs = list(range(st * st_kb, min((st + 1) * st_kb, n_kb)))
                wt = w8p.tile([128, len(kbs), n_mb * 128], F8,
                              name=f"{wname}_t{st}", tag="w8")
                wdma(wt[:], i[wname][st].rearrange("p (b c) -> p b c", b=len(kbs)))
                for bi, kb in enumerate(kbs):
                    for mb in range(n_mb):
                        nc.tensor.matmul(
                            ps[:, mb:mb + 1],
                            wt[:, bi, mb * 128:(mb + 1) * 128],
                            act8[:, kb:kb + 1],
                            start=(kb == 0 and mb == 0),
                            stop=(kb == n_kb - 1 and mb == n_mb - 1))
            return ps

        def combine(ps16, ps8, inv_sc, n_mb, tag, lo=True):
            out = sm.tile([128, n_mb], F32, name=f"cmb_{tag}")
            if lo:
                p3 = ps16[:].rearrange("p (m two) -> p m two", two=2)
                nc.vector.tensor_reduce(out[:], p3, AXL.X, ALU.add)
            else:
                nc.vector.tensor_copy(out[:], ps16[:])
            if ps8 is not None:
                t = scr.tile([128, n_mb], F32, name=f"cmb8_{tag}", tag="cmb8")
                nc.vector.tensor_scalar_mul(t[:], ps8[:], inv_sc[:])
                nc.vector.tensor_add(out[:], out[:], t[:])
            return out

        def rope_cols(cols, rot, tag):
            ps = pp.tile([128, HEADS_PC], F32, name=f"rope_{tag}", tag="ps")
            nc.tensor.matmul(ps[:], rot[:], cols[:], start=True, stop=True)
            out = sm.tile([128, HEADS_PC], F32, name=f"rot_{tag}")
            nc.vector.tensor_copy(out[:], ps[:])
            return out

        # ---- rmsnorm #1, activation splits ----
        anorm = sm.tile([32, 128], F32, name="anorm")
        nc.sync.dma_start(anorm[:], i["attn_norm"].rearrange("(a d) -> a d", a=32))
        fnorm = sm.tile([32, 128], F32, name="fnorm")
        nc.sync.dma_start(fnorm[:], i["ffn_norm"].rearrange("(a d) -> a d", a=32))

        x_rows, h_cols = rmsnorm_cols(i["x"], anorm, "a")
        h_hl = sm.tile([128, KB, 2], F16, name="h_hl")
        h_8 = sm.tile([128, KB], F8, name="h_8")
        split_hl(h_cols[:], h_hl, h_8, slice(0, KB))
        x_ps = pp.tile([128, 32], F32, name="x_ps", tag="ps")
        nc.tensor.transpose(x_ps[:], x_rows[:], ident32[:])
        x_cols = sm.tile([128, 32], F32, name="x_cols")
        nc.vector.tensor_scalar_mul(x_cols[:], x_ps[:], 1.0 / N_CORES)

        # ---- q projection + rope ----
        ps_q = gemv16("wq16", HEADS_PC, h_hl, True, 16, KB, wA, "q")
        ps_q8 = gemv8("wq8", HEADS_PC, h_8, 16, KB, "q")
        q_cols = combine(ps_q, ps_q8, inv_q, HEADS_PC, "q")
        qr_cols = rope_cols(q_cols, rot_q, "q")
        q16 = sm.tile([128, HEADS_PC], F16, name="q16")
        nc.vector.tensor_copy(q16[:], qr_cols[:])

        # ---- attention over the KV cache ----
        o_psum = pp.tile([128, HEADS_PC], F32, name="o_psum", tag="ps")
        denom_acc = sm.tile([128, HEADS_PC], F32, name="denom_acc")
        nc.vector.memset(denom_acc[:], 0.0)

        kT_t = []
        for st in range(2):
            kt = wA.tile([128, HEADS_PC, 2048], F16, name=f"kT_{st}", tag="wA")
            wdma(kt[:], i["kT16"][st].rearrange("p (h t) -> p h t", h=HEADS_PC))
            kT_t.append(kt)
        v_t = []
        for st in range(2):
            vt = wA.tile([128, 16, QKV_N], F16, name=f"v_{st}", tag="wA")
            wdma(vt[:], i["v16"][st].rearrange("p (b c) -> p b c", b=16))
            v_t.append(vt)

        def emit_scores(tt):
            st, loc = tt // 16, tt % 16
            s_ps = pp.tile([128, HEADS_PC], F32, name="s_ps", tag="ps")
            for h in range(HEADS_PC):
                nc.tensor.matmul(
                    s_ps[:, h:h + 1],
                    kT_t[st][:, h, loc * 128:(loc + 1) * 128],
                    q16[:, h:h + 1],
                    start=(h == 0), stop=(h == HEADS_PC - 1))
            expt = scr.tile([128, HEADS_PC], F32, name="expt", tag="expt")
            nc.scalar.activation(expt[:], s_ps[:], AF.Exp)
            nc.vector.tensor_add(denom_acc[:], denom_acc[:], expt[:])
            expt16 = scr.tile([128, HEADS_PC], F16, name="expt16", tag="expt16")
            nc.vector.tensor_copy(expt16[:], expt[:])
            return expt16

        pending = emit_scores(0)
        for tt in range(T_TILES):
            st, loc = tt // 16, tt % 16
            expt16 = pending
            if tt + 1 < T_TILES:
                pending = emit_scores(tt + 1)
            for h in range(HEADS_PC):
                nc.tensor.matmul(
                    o_psum[:, h:h + 1],
                    v_t[st][:, loc, h * 128:(h + 1) * 128],
                    expt16[:, h:h + 1],
                    start=(tt == 0 and h == 0),
                    stop=(tt == T_TILES - 1 and h == HEADS_PC - 1))

        # ---- k/v of the current token ----
        ps_k = gemv16("wk16", HEADS_PC, h_hl, False, 16, KB, wA, "k")
        k_cols = combine(ps_k, None, None, HEADS_PC, "k", lo=False)
        kr_cols = rope_cols(k_cols, rot_k, "k")
        ps_v = gemv16("wv16", HEADS_PC, h_hl, False, 16, KB, wA, "v")
        v_cols = combine(ps_v, None, None, HEADS_PC, "v", lo=False)

        qk = scr.tile([128, HEADS_PC], F32, name="qk", tag="qk")
        nc.vector.tensor_tensor(qk[:], qr_cols[:], kr_cols[:], ALU.mult)
        snew_ps = pp.tile([1, HEADS_PC], F32, name="snew", tag="ps")
        nc.tensor.matmul(snew_ps[:], ones128[:], qk[:], start=True, stop=True)
        e_new = sm.tile([1, HEADS_PC], F32, name="e_new")
        nc.scalar.activation(e_new[:], snew_ps[:], AF.Exp)

        denom_ps = pp.tile([1, HEADS_PC], F32, name="denom_ps", tag="ps")
        nc.tensor.matmul(denom_ps[:], ones128[:], denom_acc[:],
                         start=True, stop=True)
        denom = sm.tile([1, HEADS_PC], F32, name="denom")
        nc.vector.tensor_copy(denom[:], denom_ps[:])
        nc.vector.tensor_add(denom[:], denom[:], e_new[:])
        nc.vector.reciprocal(denom[:], denom[:])
        rec_ps = pp.tile([128, HEADS_PC], F32, name="rec_ps", tag="ps")
        nc.tensor.matmul(rec_ps[:], ones_r128[:], denom[:], start=True, stop=True)
        enew_ps = pp.tile([128, HEADS_PC], F32, name="enew_ps", tag="ps")
        nc.tensor.matmul(enew_ps[:], ones_r128[:], e_new[:], start=True, stop=True)

        o_sb = sm.tile([128, HEADS_PC], F32, name="o_sb")
        nc.vector.tensor_tensor(o_sb[:], enew_ps[:], v_cols[:], ALU.mult)
        nc.vector.tensor_add(o_sb[:], o_sb[:], o_psum[:])
        nc.vector.tensor_tensor(o_sb[:], o_sb[:], rec_ps[:], ALU.mult)
        o_hl = sm.tile([128, HEADS_PC, 2], F16, name="o_hl")
        o_8 = sm.tile([128, HEADS_PC], F8, name="o_8")
        split_hl(o_sb[:], o_hl, o_8, slice(0, HEADS_PC))

        # ---- o @ w_o + x/8 -> AR1 ----
        ps_wo = gemv16("wo16", KB, o_hl, True, 2, HEADS_PC, wA, "wo")
        ps_wo8 = gemv8("wo8", KB, o_8, 2, HEADS_PC, "wo")
        y1_cols = combine(ps_wo, ps_wo8, inv_o, KB, "wo")
        nc.vector.tensor_add(y1_cols[:], y1_cols[:], x_cols[:])
        y1r_ps = pp.tile([32, 128], F32, name="y1r", tag="ps")
        nc.tensor.transpose(y1r_ps[:], y1_cols[:], ident128[:])
        y1_rows = sm.tile([32, 128], F32, name="y1_rows")
        nc.vector.tensor_copy(y1_rows[:], y1r_ps[:])

        ar1_in = dram.tile([HIDDEN], F32, name="ar1_in")
        ar1_out = nc.dram_tensor("ar1_out", [HIDDEN], F32, kind="Internal",
                                 addr_space="Shared").ap()
        nc.scalar.dma_start(ar1_in[:].rearrange("(a d) -> a d", a=32), y1_rows[:])
        nc.gpsimd.collective_compute(
            "AllReduce", ALU.add,
            replica_groups=[list(range(N_CORES))],
            ins=[ar1_in[:].opt()], outs=[ar1_out[:].opt()],
        )

        # ---- MLP ----
        x2_rows, h2_cols = rmsnorm_cols(ar1_out[:], fnorm, "b", eng=nc.scalar)
        h2_hl = sm.tile([128, KB, 2], F16, name="h2_hl")
        h2_8 = sm.tile([128, KB], F8, name="h2_8")
        split_hl(h2_cols[:], h2_hl, h2_8, slice(0, KB))
        x2_ps = pp.tile([128, 32], F32, name="x2_ps", tag="ps")
        nc.tensor.transpose(x2_ps[:], x2_rows[:], ident32[:])
        x2_cols = sm.tile([128, 32], F32, name="x2_cols")
        nc.vector.tensor_scalar_mul(x2_cols[:], x2_ps[:], 1.0 / N_CORES)

        # ff1: column-chunked supertiles; a-blocks complete per chunk
        a_hl = sm.tile([128, 11, 2], F16, name="a_hl")
        a_8 = sm.tile([128, 11], F8, name="a_8")
        mb0 = 0
        off = 0
        for sti, nb in enumerate(FF1_STS):
            csz = nb * 128
            wt = wA.tile([128, KB, csz], F16, name=f"wf1_t{sti}", tag="wA")
            wdma(wt[:], i["wf1_16"][:, off:off + KB * csz].rearrange(
                "p (b c) -> p b c", b=KB))
            wt8 = w8p.tile([128, KB, csz], F8, name=f"wf1_8t{sti}", tag="w8")
            wdma(wt8[:], i["wf1_8"][:, off:off + KB * csz].rearrange(
                "p (b c) -> p b c", b=KB))
            off += KB * csz
            ps = pp.tile([128, 2 * nb], F32, name=f"ps16_f1_{sti}", tag="ps")
            ps8 = pp.tile([128, nb], F32, name=f"ps8_f1_{sti}", tag="ps")
            for kb in range(KB):
                for mbl in range(nb):
                    nc.tensor.matmul(
                        ps[:, 2 * mbl:2 * mbl + 2],
                        wt[:, kb, mbl * 128:(mbl + 1) * 128],
                        h2_hl[:, kb, :],
                        start=(kb == 0 and mbl == 0),
                        stop=(kb == KB - 1 and mbl == nb - 1))
            for kb in range(KB):
                for mbl in range(nb):
                    nc.tensor.matmul(
                        ps8[:, mbl:mbl + 1],
                        wt8[:, kb, mbl * 128:(mbl + 1) * 128],
                        h2_8[:, kb:kb + 1],
                        start=(kb == 0 and mbl == 0),
                        stop=(kb == KB - 1 and mbl == nb - 1))
            a_blk = combine(ps, ps8, inv_f1, nb, f"f1_{sti}")
            sig = scr.tile([128, nb], F32, name=f"sig_{sti}", tag="sig")
            nc.scalar.activation(sig[:], a_blk[:], AF.Sigmoid)
            nc.vector.tensor_tensor(a_blk[:], a_blk[:], sig[:], ALU.mult)
            if sti == len(FF1_STS) - 1:
                nc.vector.memset(a_blk[96:128, nb - 1:nb], 0.0)
            split_hl(a_blk[:], a_hl, a_8, slice(mb0, mb0 + nb))
            mb0 += nb

        # ff2: column-chunked sts with per-st psum groups + incremental
        # combine, so only the last st's matmuls trail the final DMA byte
        y2_cols = sm.tile([128, KB], F32, name="y2_cols")
        nmb = WF2_CS // 128                          # 4
        for st in range(WF2_STS):
            wt = wB.tile([128, 11, WF2_CS], F16, name=f"wf2_t{st}", tag="wB")
            wdma(wt[:], i["wf2_16"][st].rearrange("p (b c) -> p b c", b=11))
            wt8 = w8p.tile([128, 11, WF2_CS], F8, name=f"wf2_8t{st}", tag="w8")
            wdma(wt8[:], i["wf2_8"][st].rearrange("p (b c) -> p b c", b=11))
            ps = pp.tile([128, 2 * nmb], F32, name=f"ps16_f2_{st}", tag="ps")
            ps8 = pp.tile([128, nmb], F32, name=f"ps8_f2_{st}", tag="ps")
            for kb in range(11):
                for mbl in range(nmb):
                    nc.tensor.matmul(
                        ps[:, 2 * mbl:2 * mbl + 2],
                        wt[:, kb, mbl * 128:(mbl + 1) * 128],
                        a_hl[:, kb, :],
                        start=(kb == 0 and mbl == 0),
                        stop=(kb == 10 and mbl == nmb - 1))
            for kb in range(11):
                for mbl in range(nmb):
                    nc.tensor.matmul(
                        ps8[:, mbl:mbl + 1],
                        wt8[:, kb, mbl * 128:(mbl + 1) * 128],
                        a_8[:, kb:kb + 1],
                        start=(kb == 0 and mbl == 0),
                        stop=(kb == 10 and mbl == nmb - 1))
            ys = y2_cols[:, st * nmb:(st + 1) * nmb]
            p3 = ps[:].rearrange("p (m two) -> p m two", two=2)
            nc.vector.tensor_reduce(ys, p3, AXL.X, ALU.add)
            t8 = scr.tile([128, nmb], F32, name=f"c8_f2_{st}", tag="cmb8")
            nc.vector.tensor_scalar_mul(t8[:], ps8[:], inv_f2[:])
            nc.vector.tensor_add(ys, ys, t8[:])
            nc.vector.tensor_add(ys, ys, x2_cols[:, st * nmb:(st + 1) * nmb])
        y2r_ps = pp.tile([32, 128], F32, name="y2r", tag="ps")
        nc.tensor.transpose(y2r_ps[:], y2_cols[:], ident128[:])
        y2_rows = sm.tile([32, 128], F32, name="y2_rows")
        nc.vector.tensor_copy(y2_rows[:], y2r_ps[:])

        ar2_in = dram.tile([HIDDEN], F32, name="ar2_in")
        ar2_out = nc.dram_tensor("ar2_out", [HIDDEN], F32, kind="Internal",
                                 addr_space="Shared").ap()
        nc.scalar.dma_start(ar2_in[:].rearrange("(a d) -> a d", a=32), y2_rows[:])
        nc.gpsimd.collective_compute(
            "AllReduce", ALU.add,
            replica_groups=[list(range(N_CORES))],
            ins=[ar2_in[:].opt()], outs=[ar2_out[:].opt()],
        )
        nc.scalar.dma_start(y[:], ar2_out[:])


_BUILT = None


def _build():
    global _BUILT
    if _BUILT is None:
        nc = bacc.Bacc("TRN2", target_bir_lowering=False, debug=False,
                       num_devices=N_CORES)
        with tile.TileContext(nc) as tc:
            _emit(nc, tc)
        nc.compile()
        _BUILT = nc
    return _BUILT


def _rot_mats(sin, cos):
    R = np.zeros((128, 128), np.float32)
    for d in range(64):
        R[d, d] = cos[d]
        R[d + 64, d] = -sin[d]
        R[d + 64, d + 64] = cos[d]
        R[d, d + 64] = sin[d]
    return R, (R * SCALE).astype(np.float32)


def _split16_8(a):
    a = np.ascontiguousarray(a, dtype=np.float32)
    w16 = a.astype(np.float16)
    r = a - w16.astype(np.float32)
    m = float(np.abs(r).max())
    c = 192.0 / m if m > 0 else 1.0
    w8 = (r * c).astype(F8NP)
    return w16, w8, np.float32(1.0 / c)


def _row_tiles(a, st_kb):
    """[n_kb*128, C] -> [n_st, 128, st_kb*C] (per-partition contiguous)."""
    rows, C = a.shape
    n_kb = rows // 128
    n_st = n_kb // st_kb
    return np.ascontiguousarray(
        a.reshape(n_st, st_kb, 128, C).transpose(0, 2, 1, 3).reshape(
            n_st, 128, st_kb * C))


def _col_tiles_flat(a, sts):
    """[n_kb*128, C] col-chunked by sts mb-blocks -> [128, n_kb*C] concat."""
    n_kb = a.shape[0] // 128
    out = []
    mb0 = 0
    for nb in sts:
        csz = nb * 128
        blk = a[:, mb0 * 128:mb0 * 128 + csz]
        out.append(blk.reshape(n_kb, 128, csz).transpose(1, 0, 2).reshape(
            128, n_kb * csz))
        mb0 += nb
    return np.ascontiguousarray(np.concatenate(out, axis=1))


def _shard(inputs):
    f = lambda a: np.ascontiguousarray(np.asarray(a, dtype=np.float32))
    x = f(inputs["x"])
    attn_norm = f(inputs["attn_norm"])
    ffn_norm = f(inputs["ffn_norm"])
    pos = int(np.asarray(inputs["pos"]))
    sin = f(inputs["sin_cache"][pos])
    cos = f(inputs["cos_cache"][pos])
    wq, wk, wv = f(inputs["w_q"]), f(inputs["w_k"]), f(inputs["w_v"])
    wo, wf1, wf2 = f(inputs["w_o"]), f(inputs["w_ff1"]), f(inputs["w_ff2"])
    kc = f(inputs["k_cache"])
    vc = f(inputs["v_cache"])
    rot_k, rot_q = _rot_mats(sin, cos)

    in_maps = []
    for c in range(N_CORES):
        qs = slice(c * QKV_N, (c + 1) * QKV_N)
        fs = slice(c * FF_N, (c + 1) * FF_N)
        hs = slice(c * HEADS_PC, (c + 1) * HEADS_PC)
        wq16, wq8, iq = _split16_8(wq[:, qs])
        wo16, wo8, io = _split16_8(wo[qs, :])
        wf1p = np.zeros((HIDDEN, FF_NP), np.float32)
        wf1p[:, :FF_N] = wf1[:, fs]
        wf116, wf18, if1 = _split16_8(wf1p)
        wf2p = np.zeros((FF_NP, HIDDEN), np.float32)
        wf2p[:FF_N] = wf2[fs, :]
        wf216, wf28, if2 = _split16_8(wf2p)
        # kT: [h*128+d, t]; supertiles over t (2 halves), contiguous layout:
        # [st, 128(d), 4(h)*2048(t)]
        kT = kc[:, hs, :].transpose(1, 2, 0).astype(np.float16)  # [4,128,4096]
        kT_tiled = np.ascontiguousarray(
            kT.reshape(HEADS_PC, 128, 2, 2048).transpose(2, 1, 0, 3).reshape(
                2, 128, HEADS_PC * 2048))
        v2d = vc[:, hs, :].reshape(KV_LEN, QKV_N).astype(np.float16)
        # wf2: column supertiles of WF2_CS cols, 11 kb rows each
        def wf2_tiles(a):
            # [FF_NP, HIDDEN] -> [WF2_STS, 128, 11*WF2_CS]
            t = a.reshape(11, 128, WF2_STS, WF2_CS).transpose(2, 1, 0, 3)
            return np.ascontiguousarray(
                t.reshape(WF2_STS, 128, 11 * WF2_CS))
        in_maps.append({
            "x": x,
            "attn_norm": attn_norm,
            "ffn_norm": ffn_norm,
            "ident32": np.eye(32, dtype=np.float32),
            "ident128": np.eye(128, dtype=np.float32),
            "rot_k": rot_k,
            "rot_q": rot_q,
            "consts": np.array([[iq, io, if1, if2]], np.float32),
            "wq16": _row_tiles(wq16, 16),
            "wq8": _row_tiles(wq8, 16),
            "wk16": _row_tiles(wk[:, qs].astype(np.float16), 16),
            "wv16": _row_tiles(wv[:, qs].astype(np.float16), 16),
            "kT16": kT_tiled,
            "v16": _row_tiles(v2d, 16),
            "wo16": _row_tiles(wo16, 2),
            "wo8": _row_tiles(wo8, 2),
            "wf1_16": _col_tiles_flat(wf116, FF1_STS),
            "wf1_8": _col_tiles_flat(wf18, FF1_STS),
            "wf2_16": wf2_tiles(wf216),
            "wf2_8": wf2_tiles(wf28),
        })
    return in_maps


def kernel(**inputs):
    nc = _build()
    in_maps = _shard(inputs)
    res = bass_utils.run_bass_kernel_spmd(
        nc, in_maps, core_ids=list(range(N_CORES)))
    return res.results[0]["y"]
